# revision 29
# baseline (speedup 1.0000x reference)
"""CapsNet dense routing kernel for 8 Trainium2 NeuronCores.

Problem: capsule routing with 3 iterations (last skips the logit update).
  u_hat[b,n,u,v] = sum_k W[n,u,k,v] * x[b,n,k]        (B=128, N=2048, U=32, K=8, V=16)
  repeat:  c = softmax(b_logit, axis=u)
           s[b,u,v] = sum_n c[n,u] u_hat[b,n,u,v]
           v = squash(s)
           b_logit[n,u] += sum_{b,v} u_hat[b,n,u,v] v[b,u,v]

Strategy: shard n (in_caps) across the 8 cores (256 each); u_hat is never
materialized.  All (u,v) planes are stored in (v,u) column order so every
broadcast-over-v elementwise op has a unit-stride inner dim (DVE 2x mode).
  - s is one fused matmul  s[b,(vu)] = sum_(nk) xT[(nk),b] * (c*W)[(nk),(vu)]
    with a [128,512] bf16 AllReduce per iteration (iters 0/1; the last uses a
    fp32 ReduceScatter whose [16,512] shard is reinterpreted as [128,64] so
    the final squash runs on 64-wide ops, with the cross-chunk v-sum done by
    the resident block-diagonal matmul).
  - the logit update pipeline (per pair of 128-row (n,k) tiles): P = x^T v on
    the PE, a ScalarE psum->bf16 copy, q = P*W on the DVE (2x), then the
    v-sum as an in-place log-tree of strided bf16 adds (TensorReduce has no
    fast mode; the tree runs 2x), a 4-tile-grouped block-diagonal "sum over
    k" matmul into a delta PSUM, and softmax folded into the s-weights:
    exp on ScalarE straight from PSUM (b itself lives in SBUF across the two
    updates - PSUM accumulation does not survive between them), den/recip on
    the DVE, and c folded into W with two grouped broadcast multiplies.
  - the dummy 8-element AllReduce fires before anything else (no input
    dependency - it reduces garbage) so the cross-core rendezvous barrier
    and collective-stream setup overlap the input DMA and the s0 matmuls.
  - a few dummy PE matmuls anchored on each AllReduce landing tile run
    during the squash so the tensor engine enters each iteration at a
    higher p-state.
Matmul operands travel in bf16 (fp32 costs 4x PE cycles); accumulation is
fp32.  sqrt inside squash uses a bitcast fast-rsqrt with Newton steps on the
DVE; s^2 uses the ScalarE Square activation (same table set as Exp).
"""

import sys

sys.path.insert(0, "/opt/trn_rl_repo")

import ml_dtypes
import numpy as np

B, N, U, K, V, NC = 128, 2048, 32, 8, 16, 8
NSH = N // NC            # 256 in_caps per core
T = NSH * K // 128       # 16 contraction tiles of 128 (n,k) rows
UV = U * V               # 512
BU = T * U               # 512 b-logit columns (k-replicated rows)
RS_P = B // NC           # 16 output rows per core from the ReduceScatter
RSQRT_MAGIC = 0x5F3759DF

_cache = {}


def _build_program():
    import concourse.mybir as mybir
    import concourse.tile as tile
    from concourse import bacc

    fp32 = mybir.dt.float32
    bf16 = mybir.dt.bfloat16
    fp8 = mybir.dt.float8e4

    nc = bacc.Bacc(
        "TRN2", target_bir_lowering=False, debug=False, num_devices=NC
    )
    xT_d = nc.dram_tensor("xT", [128, T * B], bf16, kind="ExternalInput").ap()
    xn_d = nc.dram_tensor("xn", [B, T * 128], bf16, kind="ExternalInput").ap()
    wp_d = nc.dram_tensor("wp", [128, T * UV], bf16, kind="ExternalInput").ap()
    s2_d = nc.dram_tensor("s2", [128, 128], bf16, kind="ExternalInput").ap()
    id_d = nc.dram_tensor("idm", [128, 128], bf16, kind="ExternalInput").ap()
    vo_d = nc.dram_tensor("vout", [RS_P * 8, UV // 8], fp32, kind="ExternalOutput").ap()

    with tile.TileContext(nc) as tc:
        _body(tc, nc, mybir, fp32, bf16, xT_d, xn_d, wp_d, s2_d, id_d, vo_d)
    nc.compile()
    return nc


def _squash(nc, mybir, sm, fp32, bf16, bass, s_in, v_out, s_scale, pp, tag,
            newton=1, nu=U):
    """v_out = squash(s_in * s_scale); s_in [pp, V*nu] in (v,u) order.

    factor = sqrt(n2)/(1+n2) with n2 = scale^2 * sum_v s^2; all on DVE:
    rsqrt seed by integer bitcast, Newton steps, no ScalarE tables.
    """
    Alu = mybir.AluOpType
    sq = sm.tile([pp, nu * V], fp32, tag=f"sq{tag}")
    n2 = sm.tile([pp, nu], fp32, tag=f"n2{tag}")
    y = sm.tile([pp, nu], fp32, tag=f"y{tag}")
    t = sm.tile([pp, nu], fp32, tag=f"t{tag}")
    rd_ = sm.tile([pp, nu], fp32, tag=f"rd_{tag}")
    fac = sm.tile([pp, nu], v_out.dtype, tag=f"fac{tag}")
    nc.scalar.activation(out=sq, in_=s_in,
                         func=mybir.ActivationFunctionType.Square)
    nc.vector.reduce_sum(
        out=n2, in_=sq.rearrange("p (v u) -> p u v", u=nu),
        axis=mybir.AxisListType.X,
    )
    if s_scale != 1.0:
        nc.vector.tensor_scalar_mul(out=n2, in0=n2,
                                    scalar1=float(s_scale * s_scale))
    # y0 = bitcast(0x5F3759DF - (bitcast(n2) >> 1))
    nc.vector.tensor_scalar(
        out=y.bitcast(mybir.dt.int32), in0=n2.bitcast(mybir.dt.int32),
        scalar1=1, scalar2=None, op0=Alu.logical_shift_right,
    )
    nc.vector.tensor_scalar(
        out=y.bitcast(mybir.dt.int32), in0=y.bitcast(mybir.dt.int32),
        scalar1=-1, scalar2=RSQRT_MAGIC, op0=Alu.mult, op1=Alu.add,
    )
    for _ in range(newton):  # Newton: y *= 1.5 - 0.5*n2*y*y
        nc.vector.tensor_mul(t, y, y)
        nc.vector.tensor_mul(t, t, n2)
        nc.vector.tensor_scalar(
            out=t, in0=t, scalar1=-0.5, scalar2=1.5, op0=Alu.mult, op1=Alu.add
        )
        nc.vector.tensor_mul(y, y, t)
    # d = 1+n2 ; fac = (n2*scale)*y / d      ((n2*y) = sqrt(n2))
    nc.vector.tensor_scalar_add(out=rd_, in0=n2, scalar1=1.0)
    nc.vector.reciprocal(out=rd_, in_=rd_)
    nc.vector.scalar_tensor_tensor(
        out=t, in0=n2, scalar=float(s_scale), in1=y,
        op0=Alu.mult, op1=Alu.mult,
    )
    with nc.allow_low_precision(reason="squash factor feeds bf16 matmuls"):
        nc.vector.tensor_mul(fac, t, rd_)
    fac_b = bass.AP(
        tensor=fac.tensor, offset=fac.offset,
        ap=[fac.ap[0], [0, V], [1, nu]],
    )
    nc.vector.tensor_mul(
        v_out.rearrange("p (v u) -> p v u", u=nu),
        s_in.rearrange("p (v u) -> p v u", u=nu),
        fac_b,
    )


def _body(tc, nc, mybir, fp32, bf16, xT_d, xn_d, wp_d, s2_d, id_d, vo_d):
    from contextlib import ExitStack

    import concourse.bass as bass

    AF = mybir.ActivationFunctionType
    X = mybir.AxisListType.X
    rg = [list(range(NC))]

    ctx = ExitStack()
    tc._caps_ctx = ctx
    sing = ctx.enter_context(tc.tile_pool(name="sing", bufs=1))
    sm = ctx.enter_context(tc.tile_pool(name="sm", bufs=2))
    pbf = ctx.enter_context(tc.tile_pool(name="pbf", bufs=2))
    ps_p = ctx.enter_context(tc.tile_pool(name="ps_p", bufs=2, space="PSUM"))
    ps_s = ctx.enter_context(tc.tile_pool(name="ps_s", bufs=1, space="PSUM"))
    ps_f = ctx.enter_context(tc.tile_pool(name="ps_f", bufs=1, space="PSUM"))
    ps_b = ctx.enter_context(tc.tile_pool(name="ps_b", bufs=1, space="PSUM"))
    dram = ctx.enter_context(tc.tile_pool(name="dram", bufs=1, space="DRAM"))

    # dummy 8-element AllReduce triggered before anything else: the first
    # collective pays the cross-core rendezvous barrier + stream setup, so
    # fire it while the inputs stream in and the s0 matmuls run.
    dz_in = dram.tile([1, 8], fp32, name="dzin")
    dz_out = dram.tile([1, 8], fp32, name="dzout", addr_space="Shared")
    nc.gpsimd.collective_compute(
        "AllReduce", mybir.AluOpType.add, replica_groups=rg,
        ins=[dz_in.opt()], outs=[dz_out.opt()],
    )

    # preload the exp table set during the DMA ramp
    actwarm = sing.tile([1, 1], fp32)
    nc.vector.memset(actwarm, 1.0)
    nc.scalar.activation(out=actwarm, in_=actwarm, func=AF.Exp)

    # filler fodder: zeroed bf16 tile for p-state-pinning matmuls
    warm_sb = sing.tile([128, UV], bf16)
    nc.vector.memset(warm_sb, 0.0)
    fill_ps = ps_f.tile([128, UV], fp32)

    def fillers(n, rhs=None):
        rhs = warm_sb if rhs is None else rhs
        for _ in range(n):
            nc.tensor.matmul(
                out=fill_ps, lhsT=warm_sb[:, :128], rhs=rhs,
                start=True, stop=True, skip_group_check=True,
            )

    # ---- resident inputs ---------------------------------------------------
    xT_sb = sing.tile([128, T * B], bf16)
    xn_sb = sing.tile([B, T * 128], bf16)
    wp_sb = sing.tile([128, T * UV], bf16)
    s2_sb = sing.tile([128, 128], bf16)
    id_sb = sing.tile([128, 128], bf16)

    # interleave xT/wp chunks so s0's matmul groups start as data lands
    for c in range(4):
        xc = slice(c * 4 * B, (c + 1) * 4 * B)
        wc_ = slice(c * 4 * UV, (c + 1) * 4 * UV)
        nc.sync.dma_start(out=xT_sb[:, xc], in_=xT_d[:, xc])
        nc.scalar.dma_start(out=wp_sb[:, wc_], in_=wp_d[:, wc_])
    nc.sync.dma_start(out=s2_sb, in_=s2_d)
    nc.scalar.dma_start(out=id_sb, in_=id_d)

    def reduce_s(s_psum, it):
        """PSUM -> SBUF -> DRAM -> collective.  Returns the SBUF landing tile
        of the reduced result (bf16 AR for it<2, fp32 ReduceScatter for it=2)."""
        last = it == 2
        ar_dt = fp32 if last else bf16
        sp_d = dram.tile([B, UV], ar_dt, tag=f"spart{int(last)}",
                         name=f"spart{it}")
        s_part_sb = sm.tile([B, UV], ar_dt, tag=f"s_part{int(last)}")
        H = UV // 2
        nc.scalar.copy(out=s_part_sb[:, :H], in_=s_psum[:, :H])
        nc.sync.dma_start(out=sp_d[:, :H], in_=s_part_sb[:, :H])
        nc.scalar.copy(out=s_part_sb[:, H:], in_=s_psum[:, H:])
        nc.sync.dma_start(out=sp_d[:, H:], in_=s_part_sb[:, H:])
        if last:
            # land the [16,512] shard as [128,64] (same bytes): DVE/ScalarE
            # cost scales with the free dim, so the final squash ops run 8x
            # cheaper; the v-sum across the 8 row-chunks uses the resident
            # block-diagonal matmul
            sg_d = dram.tile([RS_P * 8, UV // 8], fp32, name="sglob2")
            nc.gpsimd.collective_compute(
                "ReduceScatter", mybir.AluOpType.add, replica_groups=rg,
                ins=[sp_d.opt()], outs=[sg_d.opt()],
            )
            s_sb = sm.tile([RS_P * 8, UV // 8], fp32, tag="s_sb2")
        else:
            sg_d = dram.tile([B, UV], bf16, tag="sglob", name=f"sglob{it}",
                             addr_space="Shared")
            nc.gpsimd.collective_compute(
                "AllReduce", mybir.AluOpType.add, replica_groups=rg,
                ins=[sp_d.opt()], outs=[sg_d.opt()],
            )
            s_sb = sm.tile([B, UV], bf16, tag="s_sb", name=f"s_sb{it}")
            nc.sync.dma_start(out=s_sb[:, :H], in_=sg_d[:, :H])
            nc.scalar.dma_start(out=s_sb[:, H:], in_=sg_d[:, H:])
            return s_sb, s_part_sb
        nc.sync.dma_start(out=s_sb, in_=sg_d)
        return s_sb, s_part_sb

    # ---- iteration 0: c uniform, s0 = sum_t xT_t.T @ wp_t ------------------
    s_psum = ps_s.tile([B, UV], fp32, tag="s")
    for t in range(T):
        nc.tensor.matmul(
            out=s_psum,
            lhsT=xT_sb[:, t * B : (t + 1) * B],
            rhs=wp_sb[:, t * UV : (t + 1) * UV],
            start=(t == 0),
            stop=(t == T - 1),
        )
    s_sb, s_part = reduce_s(s_psum, 0)
    # xn is only needed by iteration 0's P-matmuls: load it under AR0
    nc.scalar.dma_start(out=xn_sb[:, : 8 * 128], in_=xn_d[:, : 8 * 128])
    nc.sync.dma_start(out=xn_sb[:, 8 * 128 :], in_=xn_d[:, 8 * 128 :])

    # running b-logits: delta accumulates in PSUM within an iteration;
    # the SBUF copy carries it across iterations (PSUM accumulation does
    # not survive between the two logit updates)
    b_sb = sing.tile([128, BU], bf16)

    # ---- iterations with logit update --------------------------------------
    for i in range(2):
        s_scale = 1.0 / U if i == 0 else 1.0
        fillers(6, s_sb)
        v_bf = sm.tile([B, UV], bf16, tag="v_bf", name=f"v_bf{i}")
        _squash(nc, mybir, sm, fp32, bf16, bass, s_sb, v_bf, s_scale, B,
                f"i{i}", newton=1)

        q_sb = sm.tile([128, T * UV], bf16, tag="q")
        qr_bf = sm.tile([128, BU], bf16, tag="qr_bf")
        e_sb = sm.tile([128, BU], bf16, tag="e")
        rden = sm.tile([128, T], fp32, tag="rden")
        cn_sb = sm.tile([128, BU], bf16, tag="cn")
        wc_sb = sm.tile([128, T * UV], bf16, tag="wc")
        s_psum = ps_s.tile([B, UV], fp32, tag="s")

        # P (pairs on PE) -> psum->bf16 copy (ScalarE) -> q (DVE 2x);
        # v-sums: Pool per-tile for the first/last quads, DVE grouped for
        # the middle two (emitted early so bd groups 1/2 aren't starved)
        chunks = ((0,), (1,), (2, 3), (4, 5), (6, 7), (8, 9), (10, 11),
                  (12, 13), (14, 15))
        for ch in chunks:
            w = len(ch) * UV
            cw = slice(ch[0] * UV, (ch[-1] + 1) * UV)
            p_ps = ps_p.tile([128, 2 * UV], fp32, tag="p")
            for j, t in enumerate(ch):
                nc.tensor.matmul(
                    out=p_ps[:, j * UV : (j + 1) * UV],
                    lhsT=xn_sb[:, t * 128 : (t + 1) * 128],
                    rhs=v_bf,
                    start=True,
                    stop=True,
                )
            p_bf = pbf.tile([128, 2 * UV], bf16, tag="p_bf")
            nc.scalar.copy(out=p_bf[:, :w], in_=p_ps[:, :w])
            nc.vector.tensor_mul(q_sb[:, cw], p_bf[:, :w], wp_sb[:, cw])
            if (ch[-1] + 1) % 4 == 0:  # v-sum of the finished 4-tile group:
                g = ch[-1] // 4        # log-tree of strided bf16 adds
                bq = q_sb[:, g * 4 * UV : (g + 1) * 4 * UV]

                def vsl(j, vv):  # [p, tile(4), v(vv), u] view at v-slot j
                    vdim = [[U, vv]] if vv > 1 else []
                    return bass.AP(
                        tensor=bq.tensor, offset=bq.offset + j * U,
                        ap=[bq.ap[0], [UV, 4]] + vdim + [[1, U]],
                    )

                with nc.allow_low_precision(reason="qr feeds bf16 matmul"):
                    for vv in (8, 4, 2):  # fold v in place: 16 -> 2 slots
                        nc.vector.tensor_add(vsl(0, vv), vsl(0, vv),
                                             vsl(vv, vv))
                    nc.vector.tensor_add(
                        qr_bf[:, g * 4 * U : (g + 1) * 4 * U].rearrange(
                            "p (t u) -> p t u", u=U),
                        vsl(0, 1), vsl(1, 1),
                    )

        bd_ps = ps_b.tile([128, BU], fp32, tag="bd")

        def bd_exp(h):
            """k-sum of an 8-tile qr half -> delta in PSUM; iteration 0
            exps the delta directly (b was zero) and banks it to SBUF,
            iteration 1 folds the delta into the SBUF logits first."""
            cg = slice(h * 8 * U, (h + 1) * 8 * U)
            nc.tensor.matmul(
                out=bd_ps[:, cg], lhsT=s2_sb, rhs=qr_bf[:, cg],
                start=True, stop=(i == 0), skip_group_check=True,
            )
            if i == 1:
                # re-inject the previous iteration's logits on the PE: the
                # DVE is saturated here and a DVE add would stall the
                # exp -> c -> wc -> s chain behind the fold queue
                nc.tensor.matmul(
                    out=bd_ps[:, cg], lhsT=id_sb, rhs=b_sb[:, cg],
                    start=False, stop=True, skip_group_check=True,
                )
            with nc.allow_low_precision(reason="softmax weights in bf16"):
                nc.scalar.activation(out=e_sb[:, cg], in_=bd_ps[:, cg],
                                     func=AF.Exp)
            if i == 0:
                with nc.allow_low_precision(reason="b carried in bf16"):
                    nc.scalar.copy(out=b_sb[:, cg], in_=bd_ps[:, cg])

        def cfold(g):
            """softmax fold for tiles 4g..4g+3: den, recip, then
            wc = (e * 1/den) * W fused per tile on the DVE."""
            cg = slice(g * 4 * U, (g + 1) * 4 * U)
            nc.vector.reduce_sum(
                out=rden[:, 4 * g : 4 * g + 4],
                in_=e_sb[:, cg].rearrange("p (t u) -> p t u", u=U),
                axis=X,
            )
            nc.vector.reciprocal(
                out=rden[:, 4 * g : 4 * g + 4],
                in_=rden[:, 4 * g : 4 * g + 4],
            )
            gs = slice(4 * g * U, (4 * g + 4) * U)
            gw = slice(4 * g * UV, (4 * g + 4) * UV)
            rden_b = bass.AP(tensor=rden.tensor, offset=rden.offset + 4 * g,
                             ap=[rden.ap[0], [1, 4], [0, U]])
            with nc.allow_low_precision(reason="softmax weights in bf16"):
                nc.vector.tensor_mul(
                    cn_sb[:, gs].rearrange("p (t u) -> p t u", u=U),
                    e_sb[:, gs].rearrange("p (t u) -> p t u", u=U),
                    rden_b,
                )
            halves = ((0, 4),) if g < 3 else ((0, 2), (2, 4))
            for h0, h1 in halves:
                hw_ = slice((4 * g + h0) * UV, (4 * g + h1) * UV)
                cn_g = cn_sb[:, (4 * g + h0) * U : (4 * g + h1) * U]
                cn_b = bass.AP(tensor=cn_g.tensor, offset=cn_g.offset,
                               ap=[cn_g.ap[0], [U, h1 - h0], [0, V], [1, U]])
                nc.vector.tensor_mul(
                    wc_sb[:, hw_].rearrange("p (t v u) -> p t v u", v=V, u=U),
                    wp_sb[:, hw_].rearrange("p (t v u) -> p t v u", v=V, u=U),
                    cn_b,
                )

        def s_quad(qd):
            for t in range(4 * qd, 4 * qd + 4):
                nc.tensor.matmul(
                    out=s_psum,
                    lhsT=xT_sb[:, t * B : (t + 1) * B],
                    rhs=wc_sb[:, t * UV : (t + 1) * UV],
                    start=(t == 0),
                    stop=(t == T - 1),
                )

        bd_exp(0)
        cfold(0)
        s_quad(0)
        cfold(1)
        s_quad(1)
        bd_exp(1)
        cfold(2)
        s_quad(2)
        cfold(3)
        s_quad(3)

        s_sb, s_part = reduce_s(s_psum, i + 1)

    # ---- final squash on this core's ReduceScatter shard -------------------
    # s_sb is [128, 64] = row-chunks of the [16, 512] (v,u) shard: chunk c of
    # output row b holds v in {2c, 2c+1} for all u.
    Alu = mybir.AluOpType
    AFT = mybir.ActivationFunctionType
    sq2 = sm.tile([128, 64], fp32, tag="sq2")
    n2p = sm.tile([128, U], bf16, tag="n2p")
    n2ps = ps_b.tile([128, U], fp32, tag="n2ps")
    n2 = sm.tile([128, U], fp32, tag="n2f")
    y = sm.tile([128, U], fp32, tag="yf")
    t_ = sm.tile([128, U], fp32, tag="tf")
    rd_ = sm.tile([128, U], fp32, tag="rdf")
    fac = sm.tile([128, U], fp32, tag="facf")
    v_sb = sm.tile([128, 64], fp32, tag="v_sb2")
    nc.scalar.activation(out=sq2, in_=s_sb, func=AFT.Square)
    with nc.allow_low_precision(reason="n2 partial in bf16 for the matmul"):
        nc.vector.tensor_add(n2p, sq2[:, :U], sq2[:, U:])  # v-pair sum
    nc.tensor.matmul(out=n2ps, lhsT=s2_sb, rhs=n2p,       # chunk (v) sum,
                     start=True, stop=True,               # k-replicated
                     skip_group_check=True)
    nc.scalar.copy(out=n2, in_=n2ps)
    nc.vector.tensor_scalar(
        out=y.bitcast(mybir.dt.int32), in0=n2.bitcast(mybir.dt.int32),
        scalar1=1, scalar2=None, op0=Alu.logical_shift_right,
    )
    nc.vector.tensor_scalar(
        out=y.bitcast(mybir.dt.int32), in0=y.bitcast(mybir.dt.int32),
        scalar1=-1, scalar2=RSQRT_MAGIC, op0=Alu.mult, op1=Alu.add,
    )
    for _ in range(2):  # Newton
        nc.vector.tensor_mul(t_, y, y)
        nc.vector.tensor_mul(t_, t_, n2)
        nc.vector.tensor_scalar(
            out=t_, in0=t_, scalar1=-0.5, scalar2=1.5, op0=Alu.mult,
            op1=Alu.add,
        )
        nc.vector.tensor_mul(y, y, t_)
    nc.vector.tensor_scalar_add(out=rd_, in0=n2, scalar1=1.0)
    nc.vector.reciprocal(out=rd_, in_=rd_)
    nc.vector.tensor_mul(t_, n2, y)      # sqrt(n2)
    nc.vector.tensor_mul(fac, t_, rd_)
    fac_b = bass.AP(tensor=fac.tensor, offset=fac.offset,
                    ap=[fac.ap[0], [0, 2], [1, U]])
    nc.vector.tensor_mul(
        v_sb.rearrange("p (w u) -> p w u", u=U),
        s_sb.rearrange("p (w u) -> p w u", u=U),
        fac_b,
    )
    nc.sync.dma_start(out=vo_d, in_=v_sb)

    ctx.close()


def _host_prep(x, W):
    """Slice + relayout the full inputs into the 8 per-core input maps."""
    bf = ml_dtypes.bfloat16
    S2 = np.zeros((128, 128), np.float32)
    for p in range(0, 128, 8):
        S2[p : p + 8, p : p + 8] = 1.0
    S2 = S2.astype(bf)
    maps = []
    for c in range(NC):
        sl = slice(c * NSH, (c + 1) * NSH)
        Wc = np.ascontiguousarray(W[sl])                    # [256,32,8,16]
        # [(n k), (v u)] column order so broadcasts over v are unit-stride in u
        Wp = Wc.transpose(0, 2, 3, 1).reshape(NSH * K, UV)
        wp = Wp.reshape(T, 128, UV).transpose(1, 0, 2).reshape(128, T * UV)
        xc = np.ascontiguousarray(x[:, sl, :])              # [128,256,8]
        xn = xc.reshape(B, NSH * K)
        xT = (
            xc.transpose(1, 2, 0)
            .reshape(T, 128, B)
            .transpose(1, 0, 2)
            .reshape(128, T * B)
        )
        maps.append(
            {
                "xT": np.ascontiguousarray(xT).astype(bf),
                "xn": np.ascontiguousarray(xn).astype(bf),
                "wp": np.ascontiguousarray(wp).astype(bf),
                "s2": S2,
                "idm": np.eye(128, dtype=np.float32).astype(bf),
            }
        )
    return maps


def _gather(results):
    """Concatenate the per-core ReduceScatter shards and undo the (v,u)
    column order -> [B, U, V] fp32."""
    out = np.concatenate(
        [results[c]["vout"].reshape(RS_P, V, U) for c in range(NC)], axis=0
    )
    return np.ascontiguousarray(out.transpose(0, 2, 1)).astype(np.float32)


def kernel(x: np.ndarray, W: np.ndarray) -> np.ndarray:
    from concourse.bass_utils import run_bass_kernel_spmd

    if "nc" not in _cache:
        _cache["nc"] = _build_program()
    nc = _cache["nc"]
    in_maps = _host_prep(np.asarray(x, np.float32), np.asarray(W, np.float32))
    res = run_bass_kernel_spmd(nc, in_maps, core_ids=list(range(NC)))
    return _gather(res.results)


# revision 30
# speedup vs baseline: 1.0810x; 1.0810x over previous
"""CapsNet dense routing kernel for 8 Trainium2 NeuronCores.

Problem: capsule routing with 3 iterations (last skips the logit update).
  u_hat[b,n,u,v] = sum_k W[n,u,k,v] * x[b,n,k]        (B=128, N=2048, U=32, K=8, V=16)
  repeat:  c = softmax(b_logit, axis=u)
           s[b,u,v] = sum_n c[n,u] u_hat[b,n,u,v]
           v = squash(s)
           b_logit[n,u] += sum_{b,v} u_hat[b,n,u,v] v[b,u,v]

Strategy: shard n (in_caps) across the 8 cores (256 each); u_hat is never
materialized.  All (u,v) planes are stored in (v,u) column order so every
broadcast-over-v elementwise op has a unit-stride inner dim (DVE 2x mode).
  - s is one fused matmul  s[b,(vu)] = sum_(nk) xT[(nk),b] * (c*W)[(nk),(vu)]
    with a [128,512] bf16 AllReduce per iteration (iters 0/1; the last uses a
    fp32 ReduceScatter whose [16,512] shard is reinterpreted as [128,64] so
    the final squash runs on 64-wide ops, with the cross-chunk v-sum done by
    the resident block-diagonal matmul).
  - the logit update pipeline (per pair of 128-row (n,k) tiles): P = x^T v on
    the PE, a ScalarE psum->bf16 copy, q = P*W on the DVE (2x), then the
    v-sum as an in-place log-tree of strided bf16 adds (TensorReduce has no
    fast mode; the tree runs 2x), a 4-tile-grouped block-diagonal "sum over
    k" matmul into a delta PSUM, and softmax folded into the s-weights:
    exp on ScalarE straight from PSUM (b itself lives in SBUF across the two
    updates - PSUM accumulation does not survive between them), den/recip on
    the DVE, and c folded into W with two grouped broadcast multiplies.
  - the dummy 8-element AllReduce fires before anything else (no input
    dependency - it reduces garbage) so the cross-core rendezvous barrier
    and collective-stream setup overlap the input DMA and the s0 matmuls.
  - a few dummy PE matmuls anchored on each AllReduce landing tile run
    during the squash so the tensor engine enters each iteration at a
    higher p-state.
Matmul operands travel in bf16 (fp32 costs 4x PE cycles); accumulation is
fp32.  sqrt inside squash uses a bitcast fast-rsqrt with Newton steps on the
DVE; s^2 uses the ScalarE Square activation (same table set as Exp).
"""

import sys

sys.path.insert(0, "/opt/trn_rl_repo")

import ml_dtypes
import numpy as np

B, N, U, K, V, NC = 128, 2048, 32, 8, 16, 8
NSH = N // NC            # 256 in_caps per core
T = NSH * K // 128       # 16 contraction tiles of 128 (n,k) rows
UV = U * V               # 512
BU = T * U               # 512 b-logit columns (k-replicated rows)
RS_P = B // NC           # 16 output rows per core from the ReduceScatter
RSQRT_MAGIC = 0x5F3759DF

_cache = {}


def _build_program():
    import concourse.mybir as mybir
    import concourse.tile as tile
    from concourse import bacc

    fp32 = mybir.dt.float32
    bf16 = mybir.dt.bfloat16
    fp8 = mybir.dt.float8e4

    nc = bacc.Bacc(
        "TRN2", target_bir_lowering=False, debug=False, num_devices=NC
    )
    xT_d = nc.dram_tensor("xT", [128, T * B], bf16, kind="ExternalInput").ap()
    xn_d = nc.dram_tensor("xn", [B, T * 128], bf16, kind="ExternalInput").ap()
    wp_d = nc.dram_tensor("wp", [128, T * UV], bf16, kind="ExternalInput").ap()
    s2_d = nc.dram_tensor("s2", [128, 128], bf16, kind="ExternalInput").ap()
    id_d = nc.dram_tensor("idm", [128, 128], bf16, kind="ExternalInput").ap()
    vo_d = nc.dram_tensor("vout", [RS_P * 8, UV // 8], fp32, kind="ExternalOutput").ap()

    with tile.TileContext(nc) as tc:
        _body(tc, nc, mybir, fp32, bf16, xT_d, xn_d, wp_d, s2_d, id_d, vo_d)
    nc.compile()
    return nc


def _squash(nc, mybir, sm, fp32, bf16, bass, s_in, v_out, s_scale, pp, tag,
            newton=1, nu=U):
    """v_out = squash(s_in * s_scale); s_in [pp, V*nu] in (v,u) order.

    factor = sqrt(n2)/(1+n2) with n2 = scale^2 * sum_v s^2; all on DVE:
    rsqrt seed by integer bitcast, Newton steps, no ScalarE tables.
    """
    Alu = mybir.AluOpType
    sq = sm.tile([pp, nu * V], fp32, tag=f"sq{tag}")
    n2 = sm.tile([pp, nu], fp32, tag=f"n2{tag}")
    y = sm.tile([pp, nu], fp32, tag=f"y{tag}")
    t = sm.tile([pp, nu], fp32, tag=f"t{tag}")
    rd_ = sm.tile([pp, nu], fp32, tag=f"rd_{tag}")
    fac = sm.tile([pp, nu], v_out.dtype, tag=f"fac{tag}")
    nc.scalar.activation(out=sq, in_=s_in,
                         func=mybir.ActivationFunctionType.Square)
    nc.vector.reduce_sum(
        out=n2, in_=sq.rearrange("p (v u) -> p u v", u=nu),
        axis=mybir.AxisListType.X,
    )
    if s_scale != 1.0:
        nc.vector.tensor_scalar_mul(out=n2, in0=n2,
                                    scalar1=float(s_scale * s_scale))
    # y0 = bitcast(0x5F3759DF - (bitcast(n2) >> 1))
    nc.vector.tensor_scalar(
        out=y.bitcast(mybir.dt.int32), in0=n2.bitcast(mybir.dt.int32),
        scalar1=1, scalar2=None, op0=Alu.logical_shift_right,
    )
    nc.vector.tensor_scalar(
        out=y.bitcast(mybir.dt.int32), in0=y.bitcast(mybir.dt.int32),
        scalar1=-1, scalar2=RSQRT_MAGIC, op0=Alu.mult, op1=Alu.add,
    )
    for _ in range(newton):  # Newton: y *= 1.5 - 0.5*n2*y*y
        nc.vector.tensor_mul(t, y, y)
        nc.vector.tensor_mul(t, t, n2)
        nc.vector.tensor_scalar(
            out=t, in0=t, scalar1=-0.5, scalar2=1.5, op0=Alu.mult, op1=Alu.add
        )
        nc.vector.tensor_mul(y, y, t)
    # d = 1+n2 ; fac = (n2*scale)*y / d      ((n2*y) = sqrt(n2))
    nc.vector.tensor_scalar_add(out=rd_, in0=n2, scalar1=1.0)
    nc.vector.reciprocal(out=rd_, in_=rd_)
    nc.vector.scalar_tensor_tensor(
        out=t, in0=n2, scalar=float(s_scale), in1=y,
        op0=Alu.mult, op1=Alu.mult,
    )
    with nc.allow_low_precision(reason="squash factor feeds bf16 matmuls"):
        nc.vector.tensor_mul(fac, t, rd_)
    fac_b = bass.AP(
        tensor=fac.tensor, offset=fac.offset,
        ap=[fac.ap[0], [0, V], [1, nu]],
    )
    nc.vector.tensor_mul(
        v_out.rearrange("p (v u) -> p v u", u=nu),
        s_in.rearrange("p (v u) -> p v u", u=nu),
        fac_b,
    )


def _body(tc, nc, mybir, fp32, bf16, xT_d, xn_d, wp_d, s2_d, id_d, vo_d):
    from contextlib import ExitStack

    import concourse.bass as bass

    AF = mybir.ActivationFunctionType
    X = mybir.AxisListType.X
    rg = [list(range(NC))]

    ctx = ExitStack()
    tc._caps_ctx = ctx
    sing = ctx.enter_context(tc.tile_pool(name="sing", bufs=1))
    sm = ctx.enter_context(tc.tile_pool(name="sm", bufs=2))
    pbf = ctx.enter_context(tc.tile_pool(name="pbf", bufs=2))
    ps_p = ctx.enter_context(tc.tile_pool(name="ps_p", bufs=2, space="PSUM"))
    ps_s = ctx.enter_context(tc.tile_pool(name="ps_s", bufs=1, space="PSUM"))
    ps_f = ctx.enter_context(tc.tile_pool(name="ps_f", bufs=1, space="PSUM"))
    ps_b = ctx.enter_context(tc.tile_pool(name="ps_b", bufs=1, space="PSUM"))
    dram = ctx.enter_context(tc.tile_pool(name="dram", bufs=1, space="DRAM"))

    # dummy 8-element AllReduce triggered before anything else: the first
    # collective pays the cross-core rendezvous barrier + stream setup, so
    # fire it while the inputs stream in and the s0 matmuls run.
    dz_in = dram.tile([1, 8], fp32, name="dzin")
    dz_out = dram.tile([1, 8], fp32, name="dzout", addr_space="Shared")
    nc.gpsimd.collective_compute(
        "AllReduce", mybir.AluOpType.add, replica_groups=rg,
        ins=[dz_in.opt()], outs=[dz_out.opt()],
    )

    # preload the exp table set during the DMA ramp
    actwarm = sing.tile([1, 1], fp32)
    nc.vector.memset(actwarm, 1.0)
    nc.scalar.activation(out=actwarm, in_=actwarm, func=AF.Exp)

    # filler fodder: zeroed bf16 tile for p-state-pinning matmuls
    warm_sb = sing.tile([128, UV], bf16)
    nc.vector.memset(warm_sb, 0.0)
    fill_ps = ps_f.tile([128, UV], fp32)

    def fillers(n, rhs=None):
        rhs = warm_sb if rhs is None else rhs
        for _ in range(n):
            nc.tensor.matmul(
                out=fill_ps, lhsT=warm_sb[:, :128], rhs=rhs,
                start=True, stop=True, skip_group_check=True,
            )

    # ---- resident inputs ---------------------------------------------------
    xT_sb = sing.tile([128, T * B], bf16)
    xn_sb = sing.tile([B, T * 128], bf16)
    wp_sb = sing.tile([128, T * UV], bf16)
    s2_sb = sing.tile([128, 128], bf16)
    id_sb = sing.tile([128, 128], bf16)

    # interleave xT/wp chunks so s0's matmul groups start as data lands
    for c in range(4):
        xc = slice(c * 4 * B, (c + 1) * 4 * B)
        wc_ = slice(c * 4 * UV, (c + 1) * 4 * UV)
        nc.sync.dma_start(out=xT_sb[:, xc], in_=xT_d[:, xc])
        nc.scalar.dma_start(out=wp_sb[:, wc_], in_=wp_d[:, wc_])
    nc.sync.dma_start(out=s2_sb, in_=s2_d)
    nc.scalar.dma_start(out=id_sb, in_=id_d)

    def reduce_s(s_psum, it):
        """PSUM -> SBUF -> DRAM -> collective.  Returns the SBUF landing tile
        of the reduced result (bf16 AR for it<2, fp32 ReduceScatter for it=2)."""
        last = it == 2
        ar_dt = fp32 if last else bf16
        sp_d = dram.tile([B, UV], ar_dt, tag=f"spart{int(last)}",
                         name=f"spart{it}")
        s_part_sb = sm.tile([B, UV], ar_dt, tag=f"s_part{int(last)}")
        H = UV // 2
        nc.scalar.copy(out=s_part_sb[:, :H], in_=s_psum[:, :H])
        nc.sync.dma_start(out=sp_d[:, :H], in_=s_part_sb[:, :H])
        nc.scalar.copy(out=s_part_sb[:, H:], in_=s_psum[:, H:])
        nc.sync.dma_start(out=sp_d[:, H:], in_=s_part_sb[:, H:])
        if last:
            # land the [16,512] shard as [128,64] (same bytes): DVE/ScalarE
            # cost scales with the free dim, so the final squash ops run 8x
            # cheaper; the v-sum across the 8 row-chunks uses the resident
            # block-diagonal matmul
            sg_d = dram.tile([RS_P * 8, UV // 8], fp32, name="sglob2")
            nc.gpsimd.collective_compute(
                "ReduceScatter", mybir.AluOpType.add, replica_groups=rg,
                ins=[sp_d.opt()], outs=[sg_d.opt()],
            )
            s_sb = sm.tile([RS_P * 8, UV // 8], fp32, tag="s_sb2")
        else:
            sg_d = dram.tile([B, UV], bf16, tag="sglob", name=f"sglob{it}",
                             addr_space="Shared")
            nc.gpsimd.collective_compute(
                "AllReduce", mybir.AluOpType.add, replica_groups=rg,
                ins=[sp_d.opt()], outs=[sg_d.opt()],
            )
            s_sb = sm.tile([B, UV], bf16, tag="s_sb", name=f"s_sb{it}")
            nc.sync.dma_start(out=s_sb[:, :H], in_=sg_d[:, :H])
            nc.scalar.dma_start(out=s_sb[:, H:], in_=sg_d[:, H:])
            return s_sb, s_part_sb
        nc.sync.dma_start(out=s_sb, in_=sg_d)
        return s_sb, s_part_sb

    # ---- iteration 0: c uniform, s0 = sum_t xT_t.T @ wp_t ------------------
    s_psum = ps_s.tile([B, UV], fp32, tag="s")
    for t in range(T):
        nc.tensor.matmul(
            out=s_psum,
            lhsT=xT_sb[:, t * B : (t + 1) * B],
            rhs=wp_sb[:, t * UV : (t + 1) * UV],
            start=(t == 0),
            stop=(t == T - 1),
        )
    s_sb, s_part = reduce_s(s_psum, 0)
    # xn is only needed by iteration 0's P-matmuls: load it under AR0
    nc.scalar.dma_start(out=xn_sb[:, : 8 * 128], in_=xn_d[:, : 8 * 128])
    nc.sync.dma_start(out=xn_sb[:, 8 * 128 :], in_=xn_d[:, 8 * 128 :])

    # running b-logits: delta accumulates in PSUM within an iteration;
    # the SBUF copy carries it across iterations (PSUM accumulation does
    # not survive between the two logit updates)
    b_sb = sing.tile([128, BU], bf16)

    # ---- iterations with logit update --------------------------------------
    for i in range(2):
        s_scale = 1.0 / U if i == 0 else 1.0
        fillers(6, s_sb)
        v_bf = sm.tile([B, UV], bf16, tag="v_bf", name=f"v_bf{i}")
        _squash(nc, mybir, sm, fp32, bf16, bass, s_sb, v_bf, s_scale, B,
                f"i{i}", newton=1)

        q_sb = sm.tile([128, T * UV], bf16, tag="q")
        qr_bf = sm.tile([128, BU], bf16, tag="qr_bf")
        e_sb = sm.tile([128, BU], bf16, tag="e")
        rden = sm.tile([128, T], fp32, tag="rden")
        cn_sb = sm.tile([128, BU], bf16, tag="cn")
        wc_sb = sm.tile([128, T * UV], bf16, tag="wc")
        s_psum = ps_s.tile([B, UV], fp32, tag="s")

        # P (pairs on PE) -> psum->bf16 copy (ScalarE) -> q (DVE 2x);
        # v-sums: Pool per-tile for the first/last quads, DVE grouped for
        # the middle two (emitted early so bd groups 1/2 aren't starved)
        chunks = ((0,), (1,), (2, 3), (4, 5), (6, 7), (8, 9), (10, 11),
                  (12, 13), (14, 15))
        for ch in chunks:
            w = len(ch) * UV
            cw = slice(ch[0] * UV, (ch[-1] + 1) * UV)
            p_ps = ps_p.tile([128, 2 * UV], fp32, tag="p")
            for j, t in enumerate(ch):
                nc.tensor.matmul(
                    out=p_ps[:, j * UV : (j + 1) * UV],
                    lhsT=xn_sb[:, t * 128 : (t + 1) * 128],
                    rhs=v_bf,
                    start=True,
                    stop=True,
                )
            p_bf = pbf.tile([128, 2 * UV], bf16, tag="p_bf")
            nc.scalar.copy(out=p_bf[:, :w], in_=p_ps[:, :w])
            nc.vector.tensor_mul(q_sb[:, cw], p_bf[:, :w], wp_sb[:, cw])
            if (ch[-1] + 1) % 4 == 0:  # v-sum of the finished 4-tile group:
                g = ch[-1] // 4        # log-tree of strided bf16 adds
                bq = q_sb[:, g * 4 * UV : (g + 1) * 4 * UV]

                def vsl(j, vv):  # [p, tile(4), v(vv), u] view at v-slot j
                    vdim = [[U, vv]] if vv > 1 else []
                    return bass.AP(
                        tensor=bq.tensor, offset=bq.offset + j * U,
                        ap=[bq.ap[0], [UV, 4]] + vdim + [[1, U]],
                    )

                with nc.allow_low_precision(reason="qr feeds bf16 matmul"):
                    for vv in (8, 4, 2):  # fold v in place: 16 -> 2 slots
                        nc.vector.tensor_add(vsl(0, vv), vsl(0, vv),
                                             vsl(vv, vv))
                    nc.vector.tensor_add(
                        qr_bf[:, g * 4 * U : (g + 1) * 4 * U].rearrange(
                            "p (t u) -> p t u", u=U),
                        vsl(0, 1), vsl(1, 1),
                    )

        bd_ps = ps_b.tile([128, BU], fp32, tag="bd")

        def bd_exp(h):
            """k-sum of an 8-tile qr half -> delta in PSUM; iteration 0
            exps the delta directly (b was zero) and banks it to SBUF,
            iteration 1 folds the delta into the SBUF logits first."""
            cg = slice(h * 8 * U, (h + 1) * 8 * U)
            nc.tensor.matmul(
                out=bd_ps[:, cg], lhsT=s2_sb, rhs=qr_bf[:, cg],
                start=True, stop=(i == 0), skip_group_check=True,
            )
            if i == 1:
                # re-inject the previous iteration's logits on the PE: the
                # DVE is saturated here and a DVE add would stall the
                # exp -> c -> wc -> s chain behind the fold queue
                nc.tensor.matmul(
                    out=bd_ps[:, cg], lhsT=id_sb, rhs=b_sb[:, cg],
                    start=False, stop=True, skip_group_check=True,
                )
            with nc.allow_low_precision(reason="softmax weights in bf16"):
                nc.scalar.activation(out=e_sb[:, cg], in_=bd_ps[:, cg],
                                     func=AF.Exp)

        def cfold(g):
            """softmax fold for tiles 4g..4g+3: den, recip, then
            wc = (e * 1/den) * W fused per tile on the DVE."""
            cg = slice(g * 4 * U, (g + 1) * 4 * U)
            nc.vector.reduce_sum(
                out=rden[:, 4 * g : 4 * g + 4],
                in_=e_sb[:, cg].rearrange("p (t u) -> p t u", u=U),
                axis=X,
            )
            nc.vector.reciprocal(
                out=rden[:, 4 * g : 4 * g + 4],
                in_=rden[:, 4 * g : 4 * g + 4],
            )
            gs = slice(4 * g * U, (4 * g + 4) * U)
            gw = slice(4 * g * UV, (4 * g + 4) * UV)
            rden_b = bass.AP(tensor=rden.tensor, offset=rden.offset + 4 * g,
                             ap=[rden.ap[0], [1, 4], [0, U]])
            with nc.allow_low_precision(reason="softmax weights in bf16"):
                nc.vector.tensor_mul(
                    cn_sb[:, gs].rearrange("p (t u) -> p t u", u=U),
                    e_sb[:, gs].rearrange("p (t u) -> p t u", u=U),
                    rden_b,
                )
            halves = ((0, 4),) if g < 3 else ((0, 2), (2, 4))
            for h0, h1 in halves:
                hw_ = slice((4 * g + h0) * UV, (4 * g + h1) * UV)
                cn_g = cn_sb[:, (4 * g + h0) * U : (4 * g + h1) * U]
                cn_b = bass.AP(tensor=cn_g.tensor, offset=cn_g.offset,
                               ap=[cn_g.ap[0], [U, h1 - h0], [0, V], [1, U]])
                nc.vector.tensor_mul(
                    wc_sb[:, hw_].rearrange("p (t v u) -> p t v u", v=V, u=U),
                    wp_sb[:, hw_].rearrange("p (t v u) -> p t v u", v=V, u=U),
                    cn_b,
                )

        def s_quad(qd):
            for t in range(4 * qd, 4 * qd + 4):
                nc.tensor.matmul(
                    out=s_psum,
                    lhsT=xT_sb[:, t * B : (t + 1) * B],
                    rhs=wc_sb[:, t * UV : (t + 1) * UV],
                    start=(t == 0),
                    stop=(t == T - 1),
                )

        bd_exp(0)
        cfold(0)
        s_quad(0)
        cfold(1)
        s_quad(1)
        bd_exp(1)
        cfold(2)
        s_quad(2)
        cfold(3)
        s_quad(3)

        s_sb, s_part = reduce_s(s_psum, i + 1)
        if i == 0:
            # bank the logits for iteration 1 during the AllReduce window
            # (ScalarE is idle there; doing it inline delays the second
            # softmax half)
            with nc.allow_low_precision(reason="b carried in bf16"):
                nc.scalar.copy(out=b_sb[:, : BU // 2], in_=bd_ps[:, : BU // 2])
                nc.scalar.copy(out=b_sb[:, BU // 2 :], in_=bd_ps[:, BU // 2 :])

    # ---- final squash on this core's ReduceScatter shard -------------------
    # s_sb is [128, 64] = row-chunks of the [16, 512] (v,u) shard: chunk c of
    # output row b holds v in {2c, 2c+1} for all u.
    Alu = mybir.AluOpType
    AFT = mybir.ActivationFunctionType
    sq2 = sm.tile([128, 64], fp32, tag="sq2")
    n2p = sm.tile([128, U], bf16, tag="n2p")
    n2ps = ps_b.tile([128, U], fp32, tag="n2ps")
    n2 = sm.tile([128, U], fp32, tag="n2f")
    y = sm.tile([128, U], fp32, tag="yf")
    t_ = sm.tile([128, U], fp32, tag="tf")
    rd_ = sm.tile([128, U], fp32, tag="rdf")
    fac = sm.tile([128, U], fp32, tag="facf")
    v_sb = sm.tile([128, 64], fp32, tag="v_sb2")
    nc.scalar.activation(out=sq2, in_=s_sb, func=AFT.Square)
    with nc.allow_low_precision(reason="n2 partial in bf16 for the matmul"):
        nc.vector.tensor_add(n2p, sq2[:, :U], sq2[:, U:])  # v-pair sum
    nc.tensor.matmul(out=n2ps, lhsT=s2_sb, rhs=n2p,       # chunk (v) sum,
                     start=True, stop=True,               # k-replicated
                     skip_group_check=True)
    nc.scalar.copy(out=n2, in_=n2ps)
    nc.vector.tensor_scalar(
        out=y.bitcast(mybir.dt.int32), in0=n2.bitcast(mybir.dt.int32),
        scalar1=1, scalar2=None, op0=Alu.logical_shift_right,
    )
    nc.vector.tensor_scalar(
        out=y.bitcast(mybir.dt.int32), in0=y.bitcast(mybir.dt.int32),
        scalar1=-1, scalar2=RSQRT_MAGIC, op0=Alu.mult, op1=Alu.add,
    )
    for _ in range(2):  # Newton
        nc.vector.tensor_mul(t_, y, y)
        nc.vector.tensor_mul(t_, t_, n2)
        nc.vector.tensor_scalar(
            out=t_, in0=t_, scalar1=-0.5, scalar2=1.5, op0=Alu.mult,
            op1=Alu.add,
        )
        nc.vector.tensor_mul(y, y, t_)
    nc.vector.tensor_scalar_add(out=rd_, in0=n2, scalar1=1.0)
    nc.vector.reciprocal(out=rd_, in_=rd_)
    nc.vector.tensor_mul(t_, n2, y)      # sqrt(n2)
    nc.vector.tensor_mul(fac, t_, rd_)
    fac_b = bass.AP(tensor=fac.tensor, offset=fac.offset,
                    ap=[fac.ap[0], [0, 2], [1, U]])
    nc.vector.tensor_mul(
        v_sb.rearrange("p (w u) -> p w u", u=U),
        s_sb.rearrange("p (w u) -> p w u", u=U),
        fac_b,
    )
    nc.sync.dma_start(out=vo_d, in_=v_sb)

    ctx.close()


def _host_prep(x, W):
    """Slice + relayout the full inputs into the 8 per-core input maps."""
    bf = ml_dtypes.bfloat16
    S2 = np.zeros((128, 128), np.float32)
    for p in range(0, 128, 8):
        S2[p : p + 8, p : p + 8] = 1.0
    S2 = S2.astype(bf)
    maps = []
    for c in range(NC):
        sl = slice(c * NSH, (c + 1) * NSH)
        Wc = np.ascontiguousarray(W[sl])                    # [256,32,8,16]
        # [(n k), (v u)] column order so broadcasts over v are unit-stride in u
        Wp = Wc.transpose(0, 2, 3, 1).reshape(NSH * K, UV)
        wp = Wp.reshape(T, 128, UV).transpose(1, 0, 2).reshape(128, T * UV)
        xc = np.ascontiguousarray(x[:, sl, :])              # [128,256,8]
        xn = xc.reshape(B, NSH * K)
        xT = (
            xc.transpose(1, 2, 0)
            .reshape(T, 128, B)
            .transpose(1, 0, 2)
            .reshape(128, T * B)
        )
        maps.append(
            {
                "xT": np.ascontiguousarray(xT).astype(bf),
                "xn": np.ascontiguousarray(xn).astype(bf),
                "wp": np.ascontiguousarray(wp).astype(bf),
                "s2": S2,
                "idm": np.eye(128, dtype=np.float32).astype(bf),
            }
        )
    return maps


def _gather(results):
    """Concatenate the per-core ReduceScatter shards and undo the (v,u)
    column order -> [B, U, V] fp32."""
    out = np.concatenate(
        [results[c]["vout"].reshape(RS_P, V, U) for c in range(NC)], axis=0
    )
    return np.ascontiguousarray(out.transpose(0, 2, 1)).astype(np.float32)


def kernel(x: np.ndarray, W: np.ndarray) -> np.ndarray:
    from concourse.bass_utils import run_bass_kernel_spmd

    if "nc" not in _cache:
        _cache["nc"] = _build_program()
    nc = _cache["nc"]
    in_maps = _host_prep(np.asarray(x, np.float32), np.asarray(W, np.float32))
    res = run_bass_kernel_spmd(nc, in_maps, core_ids=list(range(NC)))
    return _gather(res.results)
